# revision 17
# baseline (speedup 1.0000x reference)
"""Trainium2 Bass kernel for a 3x3 stride-1 pad-1 Conv2d (NCHW).

Problem (hardcoded): x (16, 128, 128, 128) f32, K (3, 3, 128, 256) f32.
The reference reinterprets K's flat buffer as (Cin, kh, kw, Cout) and only
writes output rows/cols 0..124 (the rest of the 128x128 output stays zero).

Strategy: data-parallel over batch — 2 images per NeuronCore on 8 cores.
Per image the padded activation plane (Cin=128 partitions x 130x130) lives
in SBUF; the conv is 9 accumulated matmuls (contraction over Cin=128) per
output tile of 4 rows x 125 cols (N=500, one PSUM bank) per Cout half.
Everything runs in fp16 (10 mantissa bits, ~fp32r accuracy; half the
DMA/SBUF traffic, FWL-accelerated weight loads) with fp32 PSUM
accumulation. Results are cast to fp16 into a full 128x128 output plane in
SBUF whose border strips stay zero (two small border memsets), then DMA'd
out in 32-row chunks; the host upcasts to fp32. Input planes are loaded in
row chunks so the PE starts after ~2us instead of waiting for the full
4.3MB image.
"""

import numpy as np

import concourse.bacc as bacc
import concourse.mybir as mybir
import concourse.tile as tile
from concourse.bass_utils import run_bass_kernel_spmd

N_CORES = 8
B, CIN, H, W = 16, 128, 128, 128
COUT = 256
BPC = B // N_CORES  # images per core
HP, WP = H + 2, W + 2  # zero-padded plane
VALID = 125  # valid output rows/cols; rest is zero
F32 = mybir.dt.float32
F16 = mybir.dt.float16

# Row chunks for the input-plane DMA: small first chunk so the first
# matmuls can start early, then 32-row chunks.
X_CHUNKS = [(0, 8), (8, 40), (40, 72), (72, 104), (104, 130)]

_NC_CACHE = {}


def _build_nc(reps=1):
    nc = bacc.Bacc()
    # x arrives host-padded (130x130 planes, zero borders) in fp16.
    x_in = nc.dram_tensor("x", [BPC, CIN, HP, WP], F16, kind="ExternalInput")
    w_in = nc.dram_tensor("w", [CIN, 9 * COUT], F16, kind="ExternalInput")
    out_t = nc.dram_tensor("out", [BPC, COUT, H, W], F16, kind="ExternalOutput")

    with tile.TileContext(nc) as tc:
        with (
            tc.tile_pool(name="wpool", bufs=1) as wpool,
            tc.tile_pool(name="xpool", bufs=1) as xpool,
            tc.tile_pool(name="opool", bufs=1) as opool,
            tc.tile_pool(name="pspool", bufs=8, space="PSUM") as pspool,
        ):
            # Host lays w out as [Cin, c2, tap, 128]: each lhsT slice below
            # is a contiguous 128-column slab.
            w_sb = wpool.tile([CIN, 9 * COUT], F16)

            # One full output plane (fp16). Copies only ever write the valid
            # 125x125 region, so the border strips stay zero after these two
            # memsets and the host needs no post-processing. The memset APs
            # are disjoint from the copy APs, so they gate only the out-DMAs.
            ot = opool.tile([128, H, W], F16, tag="ot")
            nc.vector.memset(ot[:, VALID:H, :], 0.0)
            nc.vector.memset(ot[:, 0:VALID, VALID:W], 0.0)

            x_tiles = [
                xpool.tile([CIN, HP, WP], F16, tag=f"x{b}", name=f"x{b}")
                for b in range(BPC)
            ]
            nc.sync.dma_start(out=w_sb[:], in_=w_in[:])

            for rep in range(reps):
                for b in range(BPC):
                    for r0, r1 in X_CHUNKS:
                        nc.sync.dma_start(
                            out=x_tiles[b][:, r0:r1, :], in_=x_in[b, :, r0:r1, :]
                        )

                for b in range(BPC):
                    x_pad = x_tiles[b]
                    for c2 in range(2):
                        for rb in range(32):
                            r = rb * 4
                            # Last block: only row 124 is valid; N=125 not 500.
                            vr = min(4, VALID - r)
                            ps = pspool.tile([128, vr, VALID], F32, name="ps")
                            for t in range(9):
                                kh, kw = divmod(t, 3)
                                c0 = c2 * 1152 + t * 128
                                nc.tensor.matmul(
                                    ps[:],
                                    w_sb[:, c0 : c0 + 128],
                                    x_pad[:, r + kh : r + kh + vr, kw : kw + VALID],
                                    start=(t == 0),
                                    stop=(t == 8),
                                )
                            nc.vector.tensor_copy(
                                out=ot[:, r : r + vr, 0:VALID],
                                in_=ps[:],
                            )
                        # 32-row chunks; the last is split so the final DMA
                        # after the last matmul group is shorter.
                        for k0, k1 in ((0, 32), (32, 64), (64, 96), (96, 112), (112, 128)):
                            nc.sync.dma_start(
                                out=out_t[
                                    b,
                                    c2 * 128 : (c2 + 1) * 128,
                                    k0:k1,
                                    :,
                                ],
                                in_=ot[:, k0:k1, :],
                            )
    # Bacc defers register allocation and wait-splitting to compile(),
    # which finalize() runs; the SPMD exec path expects it done already.
    nc.finalize()
    return nc


def _get_nc(reps=1):
    if reps not in _NC_CACHE:
        _NC_CACHE[reps] = _build_nc(reps)
    return _NC_CACHE[reps]


def _run(x, K, trace=False, reps=1):
    x_pad = np.zeros((B, CIN, HP, WP), dtype=np.float16)
    x_pad[:, :, 1 : H + 1, 1 : W + 1] = np.asarray(x)  # fp32 -> fp16 RNE
    # Reference reinterprets K's flat buffer as (Cin, kh, kw, Cout); flat
    # (128, 2304) rows are Cin, cols are (kh*3+kw)*256 + cout. Reorder each
    # row to [c2, tap, cout128] so each Cout-half is one contiguous slab.
    w_host = np.ascontiguousarray(
        np.asarray(K, dtype=np.float32)
        .astype(np.float16)
        .reshape(CIN, 9, 2, 128)
        .transpose(0, 2, 1, 3)
    ).reshape(CIN, 9 * COUT)
    in_maps = [
        {"x": x_pad[i * BPC : (i + 1) * BPC], "w": w_host} for i in range(N_CORES)
    ]
    res = run_bass_kernel_spmd(
        _get_nc(reps), in_maps, list(range(N_CORES)), trace=trace
    )
    out = np.concatenate(
        [np.asarray(res.results[i]["out"]) for i in range(N_CORES)], axis=0
    ).astype(np.float32)
    return out, res


def kernel(x, K):
    out, _ = _run(x, K, trace=False)
    return out


# revision 22
# speedup vs baseline: 1.1956x; 1.1956x over previous
"""Trainium2 Bass kernel for a 3x3 stride-1 pad-1 Conv2d (NCHW).

Problem (hardcoded): x (16, 128, 128, 128) f32, K (3, 3, 128, 256) f32.
The reference reinterprets K's flat buffer as (Cin, kh, kw, Cout) and only
writes output rows/cols 0..124 (the rest of the 128x128 output stays zero).

Strategy: data-parallel over batch — 2 images per NeuronCore on 8 cores.
Per image the padded activation plane (Cin=128 partitions x 130x130) lives
in SBUF; the conv is 9 accumulated matmuls (contraction over Cin=128) per
output tile of 4 rows x 125 cols (N=500, one PSUM bank) per Cout half.
Everything runs in fp16 (10 mantissa bits, ~fp32r accuracy; half the
DMA/SBUF traffic, FWL-accelerated weight loads) with fp32 PSUM
accumulation. Results are cast to fp16 into one of two alternating
125x128 output planes in SBUF whose right border stays zero (one-time
memset), then DMA'd out in row chunks; the host upcasts to fp32 and
zero-pads rows 125..127. Input planes are loaded in row chunks so the PE
starts early instead of waiting for the full 4.3MB image.
"""

import numpy as np

import concourse.bacc as bacc
import concourse.mybir as mybir
import concourse.tile as tile
from concourse.bass_utils import run_bass_kernel_spmd

N_CORES = 8
B, CIN, H, W = 16, 128, 128, 128
COUT = 256
BPC = B // N_CORES  # images per core
HP, WP = H + 2, W + 2  # zero-padded plane
VALID = 125  # valid output rows/cols; rest is zero
F32 = mybir.dt.float32
F16 = mybir.dt.float16

# Row chunks for the input-plane DMA: small first chunk so the first
# matmuls can start early, then 32-row chunks.
X_CHUNKS = [(0, 8), (8, 40), (40, 72), (72, 104), (104, 130)]

_NC_CACHE = {}


def _build_nc(reps=1):
    nc = bacc.Bacc()
    # x arrives host-padded (130x130 planes, zero borders) in fp16.
    x_in = nc.dram_tensor("x", [BPC, CIN, HP, WP], F16, kind="ExternalInput")
    w_in = nc.dram_tensor("w", [CIN, 9 * COUT], F16, kind="ExternalInput")
    # Only the 125 valid output rows leave the device; the host zero-pads
    # rows 125..127 (cols 125..127 are zeroed on device).
    out_t = nc.dram_tensor("out", [BPC, COUT, VALID, W], F16, kind="ExternalOutput")

    with tile.TileContext(nc) as tc:
        with (
            tc.tile_pool(name="wpool", bufs=1) as wpool,
            tc.tile_pool(name="xpool", bufs=1) as xpool,
            tc.tile_pool(name="opool", bufs=1) as opool,
            tc.tile_pool(name="pspool", bufs=8, space="PSUM") as pspool,
        ):
            # Host lays w out as [Cin, c2, tap, 128]: each lhsT slice below
            # is a contiguous 128-column slab.
            w_sb = wpool.tile([CIN, 9 * COUT], F16)

            # Two output planes (fp16), alternating per (image, Cout-half) so
            # the next plane's PSUM->SBUF copies never WAR-wait on the
            # previous plane's out-DMAs. Copies only ever write the valid
            # 125x125 region, so cols 125..127 stay zero after the one-time
            # memset; the memset AP is disjoint from the copy APs, so it
            # gates only the out-DMAs.
            ot_tiles = [
                opool.tile([128, VALID, W], F16, tag=f"ot{i}", name=f"ot{i}")
                for i in range(2)
            ]
            for i in range(2):
                nc.vector.memset(ot_tiles[i][:, 0:VALID, VALID:W], 0.0)

            x_tiles = [
                xpool.tile([CIN, HP, WP], F16, tag=f"x{b}", name=f"x{b}")
                for b in range(BPC)
            ]
            # First x chunk before the (serial ~0.7us each) weight/chunk
            # dispatches so the first matmul's rhs is in flight earliest.
            r0, r1 = X_CHUNKS[0]
            nc.sync.dma_start(out=x_tiles[0][:, r0:r1, :], in_=x_in[0, :, r0:r1, :])
            nc.sync.dma_start(out=w_sb[:], in_=w_in[:])

            for rep in range(reps):
                for b in range(BPC):
                    skip0 = rep == 0 and b == 0
                    for r0, r1 in X_CHUNKS[1:] if skip0 else X_CHUNKS:
                        nc.sync.dma_start(
                            out=x_tiles[b][:, r0:r1, :], in_=x_in[b, :, r0:r1, :]
                        )

                for b in range(BPC):
                    x_pad = x_tiles[b]
                    for c2 in range(2):
                        ot = ot_tiles[(2 * b + c2) % 2]
                        for rb in range(32):
                            r = rb * 4
                            # Last block: only row 124 is valid; N=125 not 500.
                            vr = min(4, VALID - r)
                            ps = pspool.tile([128, vr, VALID], F32, name="ps")
                            for t in range(9):
                                kh, kw = divmod(t, 3)
                                c0 = c2 * 1152 + t * 128
                                nc.tensor.matmul(
                                    ps[:],
                                    w_sb[:, c0 : c0 + 128],
                                    x_pad[:, r + kh : r + kh + vr, kw : kw + VALID],
                                    start=(t == 0),
                                    stop=(t == 8),
                                )
                            nc.vector.tensor_copy(
                                out=ot[:, r : r + vr, 0:VALID],
                                in_=ps[:],
                            )
                        # 32-row chunks; the last is tiny so the final DMA
                        # after the last matmul group is short.
                        for k0, k1 in ((0, 32), (32, 64), (64, 96), (96, 120), (120, 125)):
                            nc.sync.dma_start(
                                out=out_t[
                                    b,
                                    c2 * 128 : (c2 + 1) * 128,
                                    k0:k1,
                                    :,
                                ],
                                in_=ot[:, k0:k1, :],
                            )
    # Bacc defers register allocation and wait-splitting to compile(),
    # which finalize() runs; the SPMD exec path expects it done already.
    nc.finalize()
    return nc


def _get_nc(reps=1):
    if reps not in _NC_CACHE:
        _NC_CACHE[reps] = _build_nc(reps)
    return _NC_CACHE[reps]


def _run(x, K, trace=False, reps=1):
    x_pad = np.zeros((B, CIN, HP, WP), dtype=np.float16)
    x_pad[:, :, 1 : H + 1, 1 : W + 1] = np.asarray(x)  # fp32 -> fp16 RNE
    # Reference reinterprets K's flat buffer as (Cin, kh, kw, Cout); flat
    # (128, 2304) rows are Cin, cols are (kh*3+kw)*256 + cout. Reorder each
    # row to [c2, tap, cout128] so each Cout-half is one contiguous slab.
    w_host = np.ascontiguousarray(
        np.asarray(K, dtype=np.float32)
        .astype(np.float16)
        .reshape(CIN, 9, 2, 128)
        .transpose(0, 2, 1, 3)
    ).reshape(CIN, 9 * COUT)
    in_maps = [
        {"x": x_pad[i * BPC : (i + 1) * BPC], "w": w_host} for i in range(N_CORES)
    ]
    res = run_bass_kernel_spmd(
        _get_nc(reps), in_maps, list(range(N_CORES)), trace=trace
    )
    # Device returns only the 125 valid rows; rows/cols 125..127 are zero.
    out = np.zeros((B, COUT, H, W), dtype=np.float32)
    out[:, :, 0:VALID, :] = np.concatenate(
        [np.asarray(res.results[i]["out"]) for i in range(N_CORES)], axis=0
    )
    return out, res


def kernel(x, K):
    out, _ = _run(x, K, trace=False)
    return out
